# revision 3
# baseline (speedup 1.0000x reference)
import sys
sys.path.insert(0, "/opt/trn_rl_repo")
import numpy as np
import concourse.bass as bass
from concourse import mybir
from concourse.bass_utils import run_bass_kernel_spmd

F32 = mybir.dt.float32
U8 = mybir.dt.uint8
ADD = mybir.AluOpType.add
MIN = mybir.AluOpType.min
MULT = mybir.AluOpType.mult
ISEQ = mybir.AluOpType.is_equal

T, B, N = 100, 64, 2048
TT = T + 1
NCORES = 8
B_LOC = B // NCORES
ELEMS = B_LOC * N
P = 128
C = 8
EROW = ELEMS // (C * P)
FIN = EROW * T
FS = EROW * TT
SENT = 3.0e38


def _build_nc():
    nc = bass.Bass()
    x_ext = nc.dram_tensor("x", [ELEMS, T], F32, kind="ExternalInput")
    r_ext = nc.dram_tensor("r", [ELEMS, T], F32, kind="ExternalInput")
    p2_ext = nc.dram_tensor("p2", [P, FS], F32, kind="ExternalInput")
    s_ext = nc.dram_tensor("s", [ELEMS, T], U8, kind="ExternalOutput")

    xv = x_ext.rearrange("(c p e) t -> c p (e t)", c=C, p=P, e=EROW)
    rv = r_ext.rearrange("(c p e) t -> c p (e t)", c=C, p=P, e=EROW)
    sv = s_ext.rearrange("(c p e) t -> c p (e t)", c=C, p=P, e=EROW)

    with (
        nc.sbuf_tensor([P, 2, EROW, T], F32) as xb,
        nc.sbuf_tensor([P, 2, EROW, T], F32) as rb,
        nc.sbuf_tensor([P, 2, EROW, TT], F32) as ub,
        nc.sbuf_tensor([P, FS], F32) as wb,
        nc.sbuf_tensor([P, 2, FS], F32) as zb,
        nc.sbuf_tensor([P, 2, EROW, T], U8) as sb,
        nc.sbuf_tensor([P, FS], F32) as p2b,
        nc.sbuf_tensor([P, FS], F32) as zerosb,
        nc.semaphore() as sem_x,
        nc.semaphore() as sem_r,
        nc.semaphore() as sem_u,
        nc.semaphore() as sem_z,
        nc.semaphore() as sem_s,
        nc.semaphore() as sem_o,
        nc.Block() as block,
    ):
        @block.sync
        def _(sync):
            sync.dma_start(p2b[:], p2_ext[:]).then_inc(sem_x, 16)
            for c in range(C):
                b = c % 2
                if c >= 2:
                    sync.wait_ge(sem_u, c - 1)
                sync.dma_start(xb[:, b], xv[c]).then_inc(sem_x, 16)

        @block.scalar
        def _(scalar):
            for c in range(C):
                b = c % 2
                if c >= 2:
                    scalar.wait_ge(sem_u, c - 1)
                scalar.dma_start(rb[:, b], rv[c]).then_inc(sem_r, 16)
                if c >= 1:
                    scalar.wait_ge(sem_s, c)
                    scalar.dma_start(sv[c - 1], sb[:, (c - 1) % 2]).then_inc(sem_o, 16)
            scalar.wait_ge(sem_s, C)
            scalar.dma_start(sv[C - 1], sb[:, (C - 1) % 2]).then_inc(sem_o, 16)

        @block.gpsimd
        def _(gpsimd):
            nc.gpsimd.memset(ub[:, 0, :, T:TT], 1.0)
            nc.gpsimd.memset(ub[:, 1, :, T:TT], 1.0)
            for c in range(C):
                b = c % 2
                gpsimd.wait_ge(sem_x, 16 * (c + 2))
                gpsimd.wait_ge(sem_r, 16 * (c + 1))
                if c >= 2:
                    gpsimd.wait_ge(sem_z, c - 1)
                nc.gpsimd.tensor_tensor(
                    ub[:, b, :, 0:T], xb[:, b], rb[:, b], ADD
                ).then_inc(sem_u, 1)
                if c >= 1:
                    gpsimd.wait_ge(sem_z, c)
                    if c >= 3:
                        gpsimd.wait_ge(sem_o, 16 * (c - 2))
                    bp = (c - 1) % 2
                    zv = zb[:, bp].rearrange("p (e t) -> p e t", e=EROW, t=TT)
                    nc.gpsimd.tensor_scalar(
                        sb[:, bp], zv[:, :, 0:T], 0.0, None, ISEQ
                    ).then_inc(sem_s, 1)
            gpsimd.wait_ge(sem_z, C)
            gpsimd.wait_ge(sem_o, 16 * (C - 2))
            bp = (C - 1) % 2
            zv = zb[:, bp].rearrange("p (e t) -> p e t", e=EROW, t=TT)
            nc.gpsimd.tensor_scalar(
                sb[:, bp], zv[:, :, 0:T], 0.0, None, ISEQ
            ).then_inc(sem_s, 1)

        @block.vector
        def _(vector):
            nc.vector.memset(zerosb[:], 0.0)
            for c in range(C):
                b = c % 2
                vector.wait_ge(sem_u, c + 1)
                if c >= 2:
                    vector.wait_ge(sem_s, c - 1)
                uflat = ub[:, b].rearrange("p a b -> p (a b)")
                nc.vector.tensor_tensor(wb[:], uflat, p2b[:], MULT)
                nc.vector.tensor_tensor_scan(
                    zb[:, b], wb[:], zerosb[:], 0.0, ADD, MIN
                ).then_inc(sem_z, 1)

    return nc


def _p2rep() -> np.ndarray:
    chain = np.empty(TT, dtype=np.float32)
    chain[:T] = (2.0 ** np.arange(T, dtype=np.float64)).astype(np.float32)
    chain[T] = SENT
    return np.ascontiguousarray(np.broadcast_to(np.tile(chain, EROW), (P, FS)))


def _make_in_maps(inp, rec):
    xt = inp.transpose(1, 2, 0)
    rt = rec.transpose(1, 2, 0)
    p2 = _p2rep()
    maps = []
    for i in range(NCORES):
        xs = np.ascontiguousarray(xt[i * B_LOC:(i + 1) * B_LOC]).reshape(ELEMS, T)
        rs = np.ascontiguousarray(rt[i * B_LOC:(i + 1) * B_LOC]).reshape(ELEMS, T)
        maps.append({"x": xs, "r": rs, "p2": p2})
    return maps


def _gather(res) -> np.ndarray:
    outs = [
        res.results[i]["s"].reshape(B_LOC, N, T).transpose(2, 0, 1)
        for i in range(NCORES)
    ]
    return np.concatenate(outs, axis=1).astype(np.float32)


def kernel(inp: np.ndarray, rec: np.ndarray) -> np.ndarray:
    inp = np.asarray(inp, dtype=np.float32)
    rec = np.asarray(rec, dtype=np.float32)
    nc = _build_nc()
    res = run_bass_kernel_spmd(nc, _make_in_maps(inp, rec), list(range(NCORES)))
    return _gather(res)


def run_traced(inp, rec, **kw):
    inp = np.asarray(inp, dtype=np.float32)
    rec = np.asarray(rec, dtype=np.float32)
    nc = _build_nc()
    return run_bass_kernel_spmd(nc, _make_in_maps(inp, rec),
                                list(range(NCORES)), trace=True, **kw)


# revision 4
# speedup vs baseline: 2.9860x; 2.9860x over previous
import sys
sys.path.insert(0, "/opt/trn_rl_repo")
import numpy as np
import concourse.bass as bass
from concourse import mybir
from concourse.bass_utils import run_bass_kernel_spmd

F32 = mybir.dt.float32
U8 = mybir.dt.uint8
ADD = mybir.AluOpType.add
MIN = mybir.AluOpType.min
MULT = mybir.AluOpType.mult
ISEQ = mybir.AluOpType.is_equal

T, B, N = 100, 64, 2048
TT = T + 1
NCORES = 8
B_LOC = B // NCORES
ELEMS = B_LOC * N
P = 128
C = 8
EROW = ELEMS // (C * P)
FIN = EROW * T
FS = EROW * TT
SENT = 3.0e38


def _build_nc():
    nc = bass.Bass()
    x_ext = nc.dram_tensor("x", [ELEMS, T], F32, kind="ExternalInput")
    r_ext = nc.dram_tensor("r", [ELEMS, T], F32, kind="ExternalInput")
    p2_ext = nc.dram_tensor("p2", [P, FS], F32, kind="ExternalInput")
    s_ext = nc.dram_tensor("s", [ELEMS, T], U8, kind="ExternalOutput")

    xv = x_ext.rearrange("(c p e) t -> c p (e t)", c=C, p=P, e=EROW)
    rv = r_ext.rearrange("(c p e) t -> c p (e t)", c=C, p=P, e=EROW)
    sv = s_ext.rearrange("(c p e) t -> c p (e t)", c=C, p=P, e=EROW)

    with (
        nc.sbuf_tensor([P, 2, EROW, T], F32) as xb,
        nc.sbuf_tensor([P, 2, EROW, T], F32) as rb,
        nc.sbuf_tensor([P, 2, EROW, TT], F32) as ub,
        nc.sbuf_tensor([P, FS], F32) as wb,
        nc.sbuf_tensor([P, 2, FS], F32) as zb,
        nc.sbuf_tensor([P, 2, EROW, T], U8) as sb,
        nc.sbuf_tensor([P, FS], F32) as p2b,
        nc.sbuf_tensor([P, FS], F32) as zerosb,
        nc.semaphore() as sem_x,
        nc.semaphore() as sem_r,
        nc.semaphore() as sem_u,
        nc.semaphore() as sem_z,
        nc.semaphore() as sem_s,
        nc.semaphore() as sem_o,
        nc.Block() as block,
    ):
        @block.sync
        def _(sync):
            sync.dma_start(p2b[:], p2_ext[:]).then_inc(sem_x, 16)
            for c in range(C):
                b = c % 2
                if c >= 2:
                    sync.wait_ge(sem_u, c - 1)
                sync.dma_start(xb[:, b], xv[c]).then_inc(sem_x, 16)
                if c >= 1:
                    sync.wait_ge(sem_s, c)
                    sync.dma_start(sv[c - 1], sb[:, (c - 1) % 2]).then_inc(sem_o, 16)
            sync.wait_ge(sem_s, C)
            sync.dma_start(sv[C - 1], sb[:, (C - 1) % 2]).then_inc(sem_o, 16)

        @block.scalar
        def _(scalar):
            for c in range(C):
                b = c % 2
                if c >= 2:
                    scalar.wait_ge(sem_u, c - 1)
                scalar.dma_start(rb[:, b], rv[c]).then_inc(sem_r, 16)
                if c >= 1:
                    scalar.wait_ge(sem_z, c)
                    if c >= 3:
                        scalar.wait_ge(sem_o, 16 * (c - 2))
                    bp = (c - 1) % 2
                    zv = zb[:, bp].rearrange("p (e t) -> p e t", e=EROW, t=TT)
                    nc.scalar.activation(
                        sb[:, bp], zv[:, :, 0:T],
                        mybir.ActivationFunctionType.Relu,
                        bias=1.0, scale=1.0e38,
                    ).then_inc(sem_s, 1)
            scalar.wait_ge(sem_z, C)
            scalar.wait_ge(sem_o, 16 * (C - 2))
            bp = (C - 1) % 2
            zv = zb[:, bp].rearrange("p (e t) -> p e t", e=EROW, t=TT)
            nc.scalar.activation(
                sb[:, bp], zv[:, :, 0:T],
                mybir.ActivationFunctionType.Relu,
                bias=1.0, scale=1.0e38,
            ).then_inc(sem_s, 1)

        @block.gpsimd
        def _(gpsimd):
            nc.gpsimd.memset(ub[:, 0, :, T:TT], 1.0)
            nc.gpsimd.memset(ub[:, 1, :, T:TT], 1.0)
            for c in range(C):
                b = c % 2
                gpsimd.wait_ge(sem_x, 16 * (c + 2))
                gpsimd.wait_ge(sem_r, 16 * (c + 1))
                if c >= 2:
                    gpsimd.wait_ge(sem_z, c - 1)
                nc.gpsimd.tensor_tensor(
                    ub[:, b, :, 0:T], xb[:, b], rb[:, b], ADD
                ).then_inc(sem_u, 1)

        @block.vector
        def _(vector):
            nc.vector.memset(zerosb[:], 0.0)
            for c in range(C):
                b = c % 2
                vector.wait_ge(sem_u, c + 1)
                if c >= 2:
                    vector.wait_ge(sem_s, c - 1)
                uflat = ub[:, b].rearrange("p a b -> p (a b)")
                nc.vector.tensor_tensor(wb[:], uflat, p2b[:], MULT)
                nc.vector.tensor_tensor_scan(
                    zb[:, b], wb[:], zerosb[:], 0.0, ADD, MIN
                ).then_inc(sem_z, 1)

    return nc


def _p2rep() -> np.ndarray:
    chain = np.empty(TT, dtype=np.float32)
    chain[:T] = (2.0 ** np.arange(T, dtype=np.float64)).astype(np.float32)
    chain[T] = SENT
    return np.ascontiguousarray(np.broadcast_to(np.tile(chain, EROW), (P, FS)))


def _make_in_maps(inp, rec):
    xt = inp.transpose(1, 2, 0)
    rt = rec.transpose(1, 2, 0)
    p2 = _p2rep()
    maps = []
    for i in range(NCORES):
        xs = np.ascontiguousarray(xt[i * B_LOC:(i + 1) * B_LOC]).reshape(ELEMS, T)
        rs = np.ascontiguousarray(rt[i * B_LOC:(i + 1) * B_LOC]).reshape(ELEMS, T)
        maps.append({"x": xs, "r": rs, "p2": p2})
    return maps


def _gather(res) -> np.ndarray:
    outs = [
        res.results[i]["s"].reshape(B_LOC, N, T).transpose(2, 0, 1)
        for i in range(NCORES)
    ]
    return np.concatenate(outs, axis=1).astype(np.float32)


def kernel(inp: np.ndarray, rec: np.ndarray) -> np.ndarray:
    inp = np.asarray(inp, dtype=np.float32)
    rec = np.asarray(rec, dtype=np.float32)
    nc = _build_nc()
    res = run_bass_kernel_spmd(nc, _make_in_maps(inp, rec), list(range(NCORES)))
    return _gather(res)


def run_traced(inp, rec, **kw):
    inp = np.asarray(inp, dtype=np.float32)
    rec = np.asarray(rec, dtype=np.float32)
    nc = _build_nc()
    return run_bass_kernel_spmd(nc, _make_in_maps(inp, rec),
                                list(range(NCORES)), trace=True, **kw)


# revision 6
# speedup vs baseline: 3.0899x; 1.0348x over previous
import sys
sys.path.insert(0, "/opt/trn_rl_repo")
import numpy as np
import concourse.bass as bass
from concourse import mybir
from concourse.bass_utils import run_bass_kernel_spmd

F32 = mybir.dt.float32
U8 = mybir.dt.uint8
ADD = mybir.AluOpType.add
MIN = mybir.AluOpType.min
MULT = mybir.AluOpType.mult
ISEQ = mybir.AluOpType.is_equal

T, B, N = 100, 64, 2048
TT = T + 1
NCORES = 8
B_LOC = B // NCORES
ELEMS = B_LOC * N
P = 128
C = 8
EROW = ELEMS // (C * P)
FIN = EROW * T
FS = EROW * TT
SENT = 3.0e38


def _build_nc():
    nc = bass.Bass()
    x_ext = nc.dram_tensor("x", [ELEMS, T], F32, kind="ExternalInput")
    r_ext = nc.dram_tensor("r", [ELEMS, T], F32, kind="ExternalInput")
    p2_ext = nc.dram_tensor("p2", [P, FS], F32, kind="ExternalInput")
    s_ext = nc.dram_tensor("s", [ELEMS, T], U8, kind="ExternalOutput")

    xv = x_ext.rearrange("(c p e) t -> c p (e t)", c=C, p=P, e=EROW)
    rv = r_ext.rearrange("(c p e) t -> c p (e t)", c=C, p=P, e=EROW)
    sv = s_ext.rearrange("(c p e) t -> c p (e t)", c=C, p=P, e=EROW)

    def gp_mult(c):
        return c % 3 == 2

    def spike(zbuf, sbuf):
        zv = zbuf.rearrange("p (e t) -> p e t", e=EROW, t=TT)
        return nc.scalar.activation(
            sbuf, zv[:, :, 0:T],
            mybir.ActivationFunctionType.Relu, bias=1.0, scale=1.0e38,
        )

    with (
        nc.sbuf_tensor([P, 4, EROW, T], F32) as xb,
        nc.sbuf_tensor([P, 4, EROW, T], F32) as rb,
        nc.sbuf_tensor([P, 3, EROW, TT], F32) as ub,
        nc.sbuf_tensor([P, 2, FS], F32) as wb,
        nc.sbuf_tensor([P, 2, FS], F32) as zb,
        nc.sbuf_tensor([P, 4, EROW, T], U8) as sb,
        nc.sbuf_tensor([P, FS], F32) as p2b,
        nc.sbuf_tensor([P, FS], F32) as zerosb,
        nc.semaphore() as sem_x,
        nc.semaphore() as sem_r,
        nc.semaphore() as sem_u,
        nc.semaphore() as sem_z,
        nc.semaphore() as sem_s,
        nc.semaphore() as sem_oe,
        nc.semaphore() as sem_oo,
        nc.Block() as block,
    ):
        @block.sync
        def _(sync):
            for c in range(C):
                if c >= 4:
                    sync.wait_ge(sem_u, c - 3)
                sync.dma_start(xb[:, c % 4], xv[c]).then_inc(sem_x, 16)
                if c == 0:
                    sync.dma_start(p2b[:], p2_ext[:]).then_inc(sem_x, 16)
                k = c - 3
                if k >= 0 and k % 2 == 0:
                    sync.wait_ge(sem_s, k + 1)
                    sync.dma_start(sv[k], sb[:, k % 4]).then_inc(sem_oe, 16)
            sync.wait_ge(sem_s, 7)
            sync.dma_start(sv[6], sb[:, 6 % 4]).then_inc(sem_oe, 16)

        @block.scalar
        def _(scalar):
            for c in range(C):
                if c >= 4:
                    scalar.wait_ge(sem_u, c - 3)
                scalar.dma_start(rb[:, c % 4], rv[c]).then_inc(sem_r, 16)
                k = c - 3
                if k >= 0 and k % 2 == 1:
                    scalar.wait_ge(sem_s, k + 1)
                    scalar.dma_start(sv[k], sb[:, k % 4]).then_inc(sem_oo, 16)
                if c >= 1:
                    scalar.wait_ge(sem_z, c)
                    j = c - 1
                    if j >= 4:
                        if (j - 4) % 2 == 0:
                            scalar.wait_ge(sem_oe, 16 * ((j - 4) // 2 + 1))
                        else:
                            scalar.wait_ge(sem_oo, 16 * ((j - 4 - 1) // 2 + 1))
                    spike(zb[:, j % 2], sb[:, j % 4]).then_inc(sem_s, 1)
            scalar.wait_ge(sem_s, 6)
            scalar.dma_start(sv[5], sb[:, 5 % 4]).then_inc(sem_oo, 16)
            scalar.wait_ge(sem_z, C)
            scalar.wait_ge(sem_oo, 16 * 2)
            spike(zb[:, (C - 1) % 2], sb[:, (C - 1) % 4]).then_inc(sem_s, 1)
            scalar.wait_ge(sem_s, C)
            scalar.dma_start(sv[7], sb[:, 7 % 4]).then_inc(sem_oo, 16)

        @block.gpsimd
        def _(gpsimd):
            for j in range(3):
                nc.gpsimd.memset(ub[:, j, :, T:TT], 1.0)
            for c in range(C):
                gpsimd.wait_ge(sem_x, 16 if c == 0 else 16 * (c + 2))
                gpsimd.wait_ge(sem_r, 16 * (c + 1))
                if c >= 3:
                    gpsimd.wait_ge(sem_z, c - 2)
                ins = nc.gpsimd.tensor_tensor(
                    ub[:, c % 3, :, 0:T], xb[:, c % 4], rb[:, c % 4], ADD
                )
                if gp_mult(c):
                    gpsimd.wait_ge(sem_z, c - 1)
                    uflat = ub[:, c % 3].rearrange("p a b -> p (a b)")
                    ins = nc.gpsimd.tensor_tensor(wb[:, c % 2], uflat, p2b[:], MULT)
                ins.then_inc(sem_u, 1)

        @block.vector
        def _(vector):
            nc.vector.memset(zerosb[:], 0.0)
            for c in range(C):
                vector.wait_ge(sem_u, c + 1)
                if c == 0:
                    vector.wait_ge(sem_x, 32)
                if c >= 2:
                    vector.wait_ge(sem_s, c - 1)
                if not gp_mult(c):
                    uflat = ub[:, c % 3].rearrange("p a b -> p (a b)")
                    nc.vector.tensor_tensor(wb[:, c % 2], uflat, p2b[:], MULT)
                nc.vector.tensor_tensor_scan(
                    zb[:, c % 2], wb[:, c % 2], zerosb[:], 0.0, ADD, MIN
                ).then_inc(sem_z, 1)

    return nc


def _p2rep() -> np.ndarray:
    chain = np.empty(TT, dtype=np.float32)
    chain[:T] = (2.0 ** np.arange(T, dtype=np.float64)).astype(np.float32)
    chain[T] = SENT
    return np.ascontiguousarray(np.broadcast_to(np.tile(chain, EROW), (P, FS)))


def _make_in_maps(inp, rec):
    xt = inp.transpose(1, 2, 0)
    rt = rec.transpose(1, 2, 0)
    p2 = _p2rep()
    maps = []
    for i in range(NCORES):
        xs = np.ascontiguousarray(xt[i * B_LOC:(i + 1) * B_LOC]).reshape(ELEMS, T)
        rs = np.ascontiguousarray(rt[i * B_LOC:(i + 1) * B_LOC]).reshape(ELEMS, T)
        maps.append({"x": xs, "r": rs, "p2": p2})
    return maps


def _gather(res) -> np.ndarray:
    outs = [
        res.results[i]["s"].reshape(B_LOC, N, T).transpose(2, 0, 1)
        for i in range(NCORES)
    ]
    return np.concatenate(outs, axis=1).astype(np.float32)


def kernel(inp: np.ndarray, rec: np.ndarray) -> np.ndarray:
    inp = np.asarray(inp, dtype=np.float32)
    rec = np.asarray(rec, dtype=np.float32)
    nc = _build_nc()
    res = run_bass_kernel_spmd(nc, _make_in_maps(inp, rec), list(range(NCORES)))
    return _gather(res)


def run_traced(inp, rec, **kw):
    inp = np.asarray(inp, dtype=np.float32)
    rec = np.asarray(rec, dtype=np.float32)
    nc = _build_nc()
    return run_bass_kernel_spmd(nc, _make_in_maps(inp, rec),
                                list(range(NCORES)), trace=True, **kw)


# revision 8
# speedup vs baseline: 3.1443x; 1.0176x over previous
import sys
sys.path.insert(0, "/opt/trn_rl_repo")
import numpy as np
import concourse.bass as bass
from concourse import mybir
from concourse.bass_utils import run_bass_kernel_spmd

F32 = mybir.dt.float32
U8 = mybir.dt.uint8
ADD = mybir.AluOpType.add
MIN = mybir.AluOpType.min
MULT = mybir.AluOpType.mult

T, B, N = 100, 64, 2048
TT = T + 1
NCORES = 8
B_LOC = B // NCORES
ELEMS = B_LOC * N
P = 128
C = 8
EROW = ELEMS // (C * P)
FS = EROW * TT
SENT = 3.0e38


def _build_nc():
    nc = bass.Bass()
    x_ext = nc.dram_tensor("x", [ELEMS, T], F32, kind="ExternalInput")
    r_ext = nc.dram_tensor("r", [ELEMS, T], F32, kind="ExternalInput")
    p2_ext = nc.dram_tensor("p2", [P, FS], F32, kind="ExternalInput")
    s_ext = nc.dram_tensor("s", [ELEMS, T], U8, kind="ExternalOutput")

    xv = x_ext.rearrange("(c p e) t -> c p (e t)", c=C, p=P, e=EROW)
    rv = r_ext.rearrange("(c p e) t -> c p (e t)", c=C, p=P, e=EROW)
    sv = s_ext.rearrange("(c p e) t -> c p (e t)", c=C, p=P, e=EROW)

    def gp_mult(c):
        return c % 3 == 2

    def spike(zbuf, sbuf):
        zv = zbuf.rearrange("p (e t) -> p e t", e=EROW, t=TT)
        return nc.scalar.activation(
            sbuf, zv[:, :, 0:T],
            mybir.ActivationFunctionType.Relu, bias=1.0, scale=1.0e38,
        )

    with (
        nc.sbuf_tensor([P, C, EROW, T], F32) as xb,
        nc.sbuf_tensor([P, C, EROW, T], F32) as rb,
        nc.sbuf_tensor([P, 3, EROW, TT], F32) as ub,
        nc.sbuf_tensor([P, 2, FS], F32) as wb,
        nc.sbuf_tensor([P, 2, FS], F32) as zb,
        nc.sbuf_tensor([P, C, EROW, T], U8) as sb,
        nc.sbuf_tensor([P, FS], F32) as p2b,
        nc.sbuf_tensor([P, FS], F32) as zerosb,
        nc.semaphore() as sem_x,
        nc.semaphore() as sem_r,
        nc.semaphore() as sem_u,
        nc.semaphore() as sem_z,
        nc.semaphore() as sem_s,
        nc.semaphore() as sem_o,
        nc.Block() as block,
    ):
        @block.sync
        def _(sync):
            for c in range(C):
                sync.dma_start(xb[:, c], xv[c]).then_inc(sem_x, 16)
                if c == 1:
                    sync.dma_start(p2b[:], p2_ext[:]).then_inc(sem_x, 16)

        @block.scalar
        def _(scalar):
            for c in range(C):
                scalar.dma_start(rb[:, c], rv[c]).then_inc(sem_r, 16)
            for k in range(C):
                scalar.wait_ge(sem_z, k + 1)
                spike(zb[:, k % 2], sb[:, k]).then_inc(sem_s, 1)
                scalar.wait_ge(sem_s, k + 1)
                scalar.dma_start(sv[k], sb[:, k]).then_inc(sem_o, 16)
            scalar.wait_ge(sem_o, 16 * C)

        @block.gpsimd
        def _(gpsimd):
            for j in range(3):
                nc.gpsimd.memset(ub[:, j, :, T:TT], 1.0)
            for c in range(C):
                gpsimd.wait_ge(sem_x, 16 * (c + 1) if c <= 1 else 16 * (c + 2))
                gpsimd.wait_ge(sem_r, 16 * (c + 1))
                if c >= 3:
                    gpsimd.wait_ge(sem_z, c - 2)
                ins = nc.gpsimd.tensor_tensor(
                    ub[:, c % 3, :, 0:T], xb[:, c], rb[:, c], ADD
                )
                if gp_mult(c):
                    gpsimd.wait_ge(sem_z, c - 1)
                    uflat = ub[:, c % 3].rearrange("p a b -> p (a b)")
                    ins = nc.gpsimd.tensor_tensor(wb[:, c % 2], uflat, p2b[:], MULT)
                ins.then_inc(sem_u, 1)

        @block.vector
        def _(vector):
            nc.vector.memset(zerosb[:], 0.0)
            for c in range(C):
                vector.wait_ge(sem_u, c + 1)
                if c == 0:
                    vector.wait_ge(sem_x, 48)
                if c >= 2:
                    vector.wait_ge(sem_s, c - 1)
                if not gp_mult(c):
                    uflat = ub[:, c % 3].rearrange("p a b -> p (a b)")
                    nc.vector.tensor_tensor(wb[:, c % 2], uflat, p2b[:], MULT)
                nc.vector.tensor_tensor_scan(
                    zb[:, c % 2], wb[:, c % 2], zerosb[:], 0.0, ADD, MIN
                ).then_inc(sem_z, 1)

    return nc


def _p2rep() -> np.ndarray:
    chain = np.empty(TT, dtype=np.float32)
    chain[:T] = (2.0 ** np.arange(T, dtype=np.float64)).astype(np.float32)
    chain[T] = SENT
    return np.ascontiguousarray(np.broadcast_to(np.tile(chain, EROW), (P, FS)))


def _make_in_maps(inp, rec):
    xt = inp.transpose(1, 2, 0)
    rt = rec.transpose(1, 2, 0)
    p2 = _p2rep()
    maps = []
    for i in range(NCORES):
        xs = np.ascontiguousarray(xt[i * B_LOC:(i + 1) * B_LOC]).reshape(ELEMS, T)
        rs = np.ascontiguousarray(rt[i * B_LOC:(i + 1) * B_LOC]).reshape(ELEMS, T)
        maps.append({"x": xs, "r": rs, "p2": p2})
    return maps


def _gather(res) -> np.ndarray:
    outs = [
        res.results[i]["s"].reshape(B_LOC, N, T).transpose(2, 0, 1)
        for i in range(NCORES)
    ]
    return np.concatenate(outs, axis=1).astype(np.float32)


def kernel(inp: np.ndarray, rec: np.ndarray) -> np.ndarray:
    inp = np.asarray(inp, dtype=np.float32)
    rec = np.asarray(rec, dtype=np.float32)
    nc = _build_nc()
    res = run_bass_kernel_spmd(nc, _make_in_maps(inp, rec), list(range(NCORES)))
    return _gather(res)


def run_traced(inp, rec, **kw):
    inp = np.asarray(inp, dtype=np.float32)
    rec = np.asarray(rec, dtype=np.float32)
    nc = _build_nc()
    return run_bass_kernel_spmd(nc, _make_in_maps(inp, rec),
                                list(range(NCORES)), trace=True, **kw)


# revision 9
# speedup vs baseline: 3.8856x; 1.2358x over previous
import sys
sys.path.insert(0, "/opt/trn_rl_repo")
import numpy as np
import concourse.bass as bass
from concourse import mybir
from concourse.bass_utils import run_bass_kernel_spmd

F32 = mybir.dt.float32
U8 = mybir.dt.uint8
ADD = mybir.AluOpType.add
MIN = mybir.AluOpType.min
MULT = mybir.AluOpType.mult

T, B, N = 100, 64, 2048
TT = T + 1
NCORES = 8
B_LOC = B // NCORES
ELEMS = B_LOC * N
P = 128
C = 8
EROW = ELEMS // (C * P)
FIN = EROW * T
FS = EROW * TT
SENT = 3.0e38


def _build_nc():
    nc = bass.Bass()
    x_ext = nc.dram_tensor("x", [ELEMS, T], F32, kind="ExternalInput")
    r_ext = nc.dram_tensor("r", [ELEMS, T], F32, kind="ExternalInput")
    p2_ext = nc.dram_tensor("p2", [P, FIN], F32, kind="ExternalInput")
    s_ext = nc.dram_tensor("s", [ELEMS, T], U8, kind="ExternalOutput")

    xv = x_ext.rearrange("(c p e) t -> c p (e t)", c=C, p=P, e=EROW)
    rv = r_ext.rearrange("(c p e) t -> c p (e t)", c=C, p=P, e=EROW)
    sv = s_ext.rearrange("(c p e) t -> c p (e t)", c=C, p=P, e=EROW)

    with (
        nc.sbuf_tensor([P, C, EROW, T], F32) as ub,
        nc.sbuf_tensor([P, 2, EROW, TT], F32) as wb,
        nc.sbuf_tensor([P, 2, FS], F32) as zb,
        nc.sbuf_tensor([P, C, EROW, T], U8) as sb,
        nc.sbuf_tensor([P, EROW, T], F32) as p2b,
        nc.sbuf_tensor([P, 1], F32) as zer1,
        nc.semaphore() as sem_x,
        nc.semaphore() as sem_u,
        nc.semaphore() as sem_z,
        nc.semaphore() as sem_s,
        nc.semaphore() as sem_o,
        nc.Block() as block,
    ):
        @block.sync
        def _(sync):
            for c in range(C):
                sync.dma_start(ub[:, c], xv[c]).then_inc(sem_x, 16)
                if c == 1:
                    sync.dma_start(p2b[:], p2_ext[:]).then_inc(sem_x, 16)

        @block.gpsimd
        def _(gpsimd):
            for c in range(C):
                gpsimd.wait_ge(sem_x, 16 * (c + 1) if c <= 1 else 16 * (c + 2))
                gpsimd.dma_start(ub[:, c], rv[c], accum_op=ADD).then_inc(sem_u, 16)

        @block.vector
        def _(vector):
            nc.vector.memset(zer1[:], 0.0)
            nc.vector.memset(wb[:, 0, :, T:TT], SENT)
            nc.vector.memset(wb[:, 1, :, T:TT], SENT)
            for c in range(C):
                vector.wait_ge(sem_u, 16 * (c + 1))
                if c == 0:
                    vector.wait_ge(sem_x, 48)
                if c >= 2:
                    vector.wait_ge(sem_s, c - 1)
                nc.vector.tensor_tensor(
                    wb[:, c % 2, :, 0:T], ub[:, c], p2b[:], MULT
                )
                nc.vector.tensor_tensor_scan(
                    zb[:, c % 2],
                    wb[:, c % 2].rearrange("p a b -> p (a b)"),
                    zer1[:].broadcast_to((P, FS)),
                    0.0, ADD, MIN,
                ).then_inc(sem_z, 1)

        @block.scalar
        def _(scalar):
            for k in range(C):
                scalar.wait_ge(sem_z, k + 1)
                zv = zb[:, k % 2].rearrange("p (e t) -> p e t", e=EROW, t=TT)
                nc.scalar.activation(
                    sb[:, k], zv[:, :, 0:T],
                    mybir.ActivationFunctionType.Relu, bias=1.0, scale=1.0e38,
                ).then_inc(sem_s, 1)
                scalar.wait_ge(sem_s, k + 1)
                scalar.dma_start(sv[k], sb[:, k]).then_inc(sem_o, 16)
            scalar.wait_ge(sem_o, 16 * C)

    return nc


def _p2rep() -> np.ndarray:
    chain = (2.0 ** np.arange(T, dtype=np.float64)).astype(np.float32)
    return np.ascontiguousarray(np.broadcast_to(np.tile(chain, EROW), (P, FIN)))


def _make_in_maps(inp, rec):
    xt = inp.transpose(1, 2, 0)
    rt = rec.transpose(1, 2, 0)
    p2 = _p2rep()
    maps = []
    for i in range(NCORES):
        xs = np.ascontiguousarray(xt[i * B_LOC:(i + 1) * B_LOC]).reshape(ELEMS, T)
        rs = np.ascontiguousarray(rt[i * B_LOC:(i + 1) * B_LOC]).reshape(ELEMS, T)
        maps.append({"x": xs, "r": rs, "p2": p2})
    return maps


def _gather(res) -> np.ndarray:
    outs = [
        res.results[i]["s"].reshape(B_LOC, N, T).transpose(2, 0, 1)
        for i in range(NCORES)
    ]
    return np.concatenate(outs, axis=1).astype(np.float32)


def kernel(inp: np.ndarray, rec: np.ndarray) -> np.ndarray:
    inp = np.asarray(inp, dtype=np.float32)
    rec = np.asarray(rec, dtype=np.float32)
    nc = _build_nc()
    res = run_bass_kernel_spmd(nc, _make_in_maps(inp, rec), list(range(NCORES)))
    return _gather(res)


def run_traced(inp, rec, **kw):
    inp = np.asarray(inp, dtype=np.float32)
    rec = np.asarray(rec, dtype=np.float32)
    nc = _build_nc()
    return run_bass_kernel_spmd(nc, _make_in_maps(inp, rec),
                                list(range(NCORES)), trace=True, **kw)


# revision 10
# speedup vs baseline: 4.0681x; 1.0469x over previous
import sys
sys.path.insert(0, "/opt/trn_rl_repo")
import numpy as np
import concourse.bass as bass
from concourse import mybir
from concourse.bass_utils import run_bass_kernel_spmd

F32 = mybir.dt.float32
U8 = mybir.dt.uint8
ADD = mybir.AluOpType.add
MIN = mybir.AluOpType.min
MULT = mybir.AluOpType.mult

T, B, N = 100, 64, 2048
TT = T + 1
NCORES = 8
B_LOC = B // NCORES
ELEMS = B_LOC * N
P = 128
C = 8
EROW = ELEMS // (C * P)
FIN = EROW * T
FS = EROW * TT
SENT = 3.0e38


def _build_nc():
    nc = bass.Bass()
    x_ext = nc.dram_tensor("x", [ELEMS, T], F32, kind="ExternalInput")
    r_ext = nc.dram_tensor("r", [ELEMS, T], F32, kind="ExternalInput")
    p2_ext = nc.dram_tensor("p2", [P, FIN], F32, kind="ExternalInput")
    s_ext = nc.dram_tensor("s", [ELEMS, T], U8, kind="ExternalOutput")

    xv = x_ext.rearrange("(c p e) t -> c p (e t)", c=C, p=P, e=EROW)
    rv = r_ext.rearrange("(c p e) t -> c p (e t)", c=C, p=P, e=EROW)
    sv = s_ext.rearrange("(c p e) t -> c p (e t)", c=C, p=P, e=EROW)

    NW = 2

    with (
        nc.sbuf_tensor([P, C, EROW, T], F32) as ub,
        nc.sbuf_tensor([P, NW, EROW, T], F32) as rb,
        nc.sbuf_tensor([P, 2, EROW, TT], F32) as wb,
        nc.sbuf_tensor([P, 2, FS], F32) as zb,
        nc.sbuf_tensor([P, C, EROW, T], U8) as sb,
        nc.sbuf_tensor([P, EROW, T], F32) as p2b,
        nc.sbuf_tensor([P, 1], F32) as zer1,
        nc.semaphore() as sem_x,
        nc.semaphore() as sem_r,
        nc.semaphore() as sem_u,
        nc.semaphore() as sem_z,
        nc.semaphore() as sem_s,
        nc.semaphore() as sem_o,
        nc.Block() as block,
    ):
        @block.sync
        def _(sync):
            for c in range(C):
                sync.dma_start(ub[:, c], xv[c]).then_inc(sem_x, 16)
                if c == 0:
                    sync.dma_start(p2b[:], p2_ext[:]).then_inc(sem_x, 16)

        @block.gpsimd
        def _(gpsimd):
            for c in range(NW, C):
                gpsimd.wait_ge(sem_x, 16 * (c + 2))
                gpsimd.dma_start(ub[:, c], rv[c], accum_op=ADD).then_inc(sem_u, 16)

        @block.vector
        def _(vector):
            nc.vector.memset(zer1[:], 0.0)
            nc.vector.memset(wb[:, 0, :, T:TT], SENT)
            nc.vector.memset(wb[:, 1, :, T:TT], SENT)
            for c in range(C):
                if c < NW:
                    vector.wait_ge(sem_x, 32 if c == 0 else 16 * (c + 2))
                    vector.wait_ge(sem_r, 16 * (c + 1))
                    nc.vector.tensor_tensor(
                        ub[:, c], ub[:, c], rb[:, c], ADD
                    )
                else:
                    vector.wait_ge(sem_u, 16 * (c - NW + 1))
                if c >= 2:
                    vector.wait_ge(sem_s, c - 1)
                nc.vector.tensor_tensor(
                    wb[:, c % 2, :, 0:T], ub[:, c], p2b[:], MULT
                )
                nc.vector.tensor_tensor_scan(
                    zb[:, c % 2],
                    wb[:, c % 2].rearrange("p a b -> p (a b)"),
                    zer1[:].broadcast_to((P, FS)),
                    0.0, ADD, MIN,
                ).then_inc(sem_z, 1)

        @block.scalar
        def _(scalar):
            for c in range(NW):
                scalar.dma_start(rb[:, c], rv[c]).then_inc(sem_r, 16)
            for k in range(C):
                scalar.wait_ge(sem_z, k + 1)
                zv = zb[:, k % 2].rearrange("p (e t) -> p e t", e=EROW, t=TT)
                nc.scalar.activation(
                    sb[:, k], zv[:, :, 0:T],
                    mybir.ActivationFunctionType.Relu, bias=1.0, scale=1.0e38,
                ).then_inc(sem_s, 1)
                scalar.wait_ge(sem_s, k + 1)
                scalar.dma_start(sv[k], sb[:, k]).then_inc(sem_o, 16)
            scalar.wait_ge(sem_o, 16 * C)

    return nc


def _p2rep() -> np.ndarray:
    chain = (2.0 ** np.arange(T, dtype=np.float64)).astype(np.float32)
    return np.ascontiguousarray(np.broadcast_to(np.tile(chain, EROW), (P, FIN)))


def _make_in_maps(inp, rec):
    xt = inp.transpose(1, 2, 0)
    rt = rec.transpose(1, 2, 0)
    p2 = _p2rep()
    maps = []
    for i in range(NCORES):
        xs = np.ascontiguousarray(xt[i * B_LOC:(i + 1) * B_LOC]).reshape(ELEMS, T)
        rs = np.ascontiguousarray(rt[i * B_LOC:(i + 1) * B_LOC]).reshape(ELEMS, T)
        maps.append({"x": xs, "r": rs, "p2": p2})
    return maps


def _gather(res) -> np.ndarray:
    outs = [
        res.results[i]["s"].reshape(B_LOC, N, T).transpose(2, 0, 1)
        for i in range(NCORES)
    ]
    return np.concatenate(outs, axis=1).astype(np.float32)


def kernel(inp: np.ndarray, rec: np.ndarray) -> np.ndarray:
    inp = np.asarray(inp, dtype=np.float32)
    rec = np.asarray(rec, dtype=np.float32)
    nc = _build_nc()
    res = run_bass_kernel_spmd(nc, _make_in_maps(inp, rec), list(range(NCORES)))
    return _gather(res)


def run_traced(inp, rec, **kw):
    inp = np.asarray(inp, dtype=np.float32)
    rec = np.asarray(rec, dtype=np.float32)
    nc = _build_nc()
    return run_bass_kernel_spmd(nc, _make_in_maps(inp, rec),
                                list(range(NCORES)), trace=True, **kw)
